# revision 50
# baseline (speedup 1.0000x reference)
"""Trainium2 Bass kernel for nn_CausalSelfAttention (B=1, T=2048, D=1024, H=16).

Sharding: 2 heads per core across 8 cores (tensor parallel). Wq/Wk/Wv
column-sharded by head, attention fully local, Wo row-sharded; host sums the
8 partial outputs (the all-reduce of the unshard step).

Per-core pipeline (all matmuls fp16 operands):
  P1  fused QKV: psq[t,384] = sum_i xT_blk.T @ [WqT|WkT|(1-l)WvT]; whole-tile
      x DMAs (2KB/partition, contiguous). RMS stats (Square on ACT from PSUM +
      segmented reduce -> rsqrt bit-trick, 1 Newton iter, 0.125 folded into
      k's scale), RoPE with the swap absorbed into strided views, v-blend
      (+lam*vi) via DVE scalar_tensor_tensor straight from PSUM.
  P2  qT/kT via xbar DMA-transpose (no PE, no DVE evac).
  P3  attention per duo-unit covering BOTH heads: 4 score matmuls into fp16
      PSUM with h0 (rows 0-63) / h1 (rows 64-127) pairs adjacent so the PE
      row-groups run concurrently; Exp (ACT) per head; tri-mask on diagonal
      blocks (DVE/Pool alternating); P@V lags one unit; lhsT=[v|1]
      accumulates L in row 64. QKV tiles for the next window, the previous
      window's out-projection, and the next chain/transposes are interleaved
      into the unit stream so the PE never idles (keeps HAM warm).
  P4  scale = 1/(L + e^sink): row-64 + e^sink -> K=2 head-indicator matmul
      broadcast -> reciprocal -> scale mult; out-proj per 128-row block into
      fp16 PSUM, fp16 evacuation alternating DVE/ACT, DMA out on sync/gpsimd.
"""

import sys

if "/opt/trn_rl_repo" not in sys.path:
    sys.path.insert(0, "/opt/trn_rl_repo")

import numpy as np
from contextlib import ExitStack

from concourse import bacc, tile
from concourse import mybir
from concourse.bass_utils import run_bass_kernel_spmd

F32 = mybir.dt.float32
F16 = mybir.dt.float16
I32 = mybir.dt.int32
AF = mybir.ActivationFunctionType
ALU = mybir.AluOpType
AX = mybir.AxisListType

T = 2048
D = 1024
HD = 64
NT = T // 128  # 16 t-tiles
RMS_EPS = float(np.finfo(np.float32).eps)


def _build_program():
    nc = bacc.Bacc("TRN2", target_bir_lowering=False, debug=False, num_devices=8)

    d_xtb = nc.dram_tensor("xtb", [NT, 128, 8, 128], F16, kind="ExternalInput").ap()
    d_wqkv = nc.dram_tensor("wqkv", [128, 8, 384], F16, kind="ExternalInput").ap()
    d_vis = nc.dram_tensor("vis", [128, NT, 128], F16, kind="ExternalInput").ap()
    d_cc = nc.dram_tensor("cc", [128, NT, 64], F16, kind="ExternalInput").ap()
    d_sc = nc.dram_tensor("sc", [128, NT, 64], F16, kind="ExternalInput").ap()
    d_wo = nc.dram_tensor("wo", [128, D], F16, kind="ExternalInput").ap()
    d_tri = nc.dram_tensor("tri", [128, 128], F16, kind="ExternalInput").ap()
    d_idn = nc.dram_tensor("idn", [128, 128], F16, kind="ExternalInput").ap()
    d_esk = nc.dram_tensor("esk", [1, 2], F16, kind="ExternalInput").ap()
    d_hind = nc.dram_tensor("hind", [1, 256], F16, kind="ExternalInput").ap()
    d_out = nc.dram_tensor("out", [D, T], F16, kind="ExternalOutput").ap()

    with tile.TileContext(nc) as tc, ExitStack() as ctx:
        sb = ctx.enter_context(tc.tile_pool(name="sb", bufs=1))
        sb_x = ctx.enter_context(tc.tile_pool(name="sb_x", bufs=4))
        sb_w1 = ctx.enter_context(tc.tile_pool(name="sb_w1", bufs=3))
        sb_w2 = ctx.enter_context(tc.tile_pool(name="sb_w2", bufs=3))
        sb_e = ctx.enter_context(tc.tile_pool(name="sb_e", bufs=6))
        sb_o = ctx.enter_context(tc.tile_pool(name="sb_o", bufs=3))
        sb_m = ctx.enter_context(tc.tile_pool(name="sb_m", bufs=3))
        # PSUM budget (8 banks): st scores f32 [128,1024] = 2 banks x 2 bufs;
        # ps_y the two per-window y accumulators; ps_q a shared 2-slot pool
        # for psq (QKV), pso/mbp (out-proj) and ptr (transposes).
        # 4 + 2 + 2 = 8 banks.
        ps_s = ctx.enter_context(tc.tile_pool(name="ps_s", bufs=2, space="PSUM"))
        ps_q = ctx.enter_context(tc.tile_pool(name="ps_q", bufs=2, space="PSUM"))
        ps_y = ctx.enter_context(tc.tile_pool(name="ps_y", bufs=2, space="PSUM"))

        # weights first on the sync queue (needed by the first matmul);
        # other constants go via the gpsimd queue so they don't delay x.
        wqkv = sb.tile([128, 8, 384], F16)
        wq_dmas = []
        for _wi in range(2):
            wq_dmas.append(
                nc.sync.dma_start(
                    out=wqkv[:, 4 * _wi : 4 * _wi + 4, :],
                    in_=d_wqkv[:, 4 * _wi : 4 * _wi + 4, :],
                )
            )
        vi_t = sb.tile([128, NT, 128], F16)
        cc_t = sb.tile([128, NT, 64], F16)
        sc_t = sb.tile([128, NT, 64], F16)
        wo = sb.tile([128, D], F16)
        esk = sb.tile([1, 2], F16)
        hind = sb.tile([1, 256], F16)
        const_dmas = []
        early_dmas = []
        early_dmas.append(nc.gpsimd.dma_start(out=esk[:], in_=d_esk[:]))
        tri = sb.tile([128, 128], F16)
        early_dmas.append(nc.gpsimd.dma_start(out=tri[:], in_=d_tri[:]))
        idn = sb.tile([128, 128], F16)
        early_dmas.append(nc.gpsimd.dma_start(out=idn[:], in_=d_idn[:]))
        const_dmas.append(nc.gpsimd.dma_start(out=wo[:], in_=d_wo[:]))
        const_dmas.append(nc.gpsimd.dma_start(out=hind[:], in_=d_hind[:]))

        stats = sb.tile([128, 64], F32)
        rbuf = sb.tile([128, 64], F32)
        qT = sb.tile([128, T], F16)
        kT = sb.tile([128, T], F16)
        vtiles = [sb.tile([128, 130], F16, tag=f"v{i}", name=f"v{i}") for i in range(NT)]
        qkr = [sb.tile([128, 256], F16, tag=f"qkr{i}", name=f"qkr{i}") for i in range(NT)]
        qkro = [sb.tile([128, 256], F16, tag=f"qkro{i}", name=f"qkro{i}") for i in range(NT)]
        yts = sb.tile([128, T], F16)

        # warm-up junk buffer first so the HAM warm-up matmuls start ASAP,
        # then the ones columns of the [vA|1|vB|1] tiles (written once)
        wz = sb.tile([128, 512], F16)
        nc.gpsimd.memset(wz[:], 0.0)
        for i in range(NT):
            nc.gpsimd.memset(
                vtiles[i][:].rearrange("p (h c) -> p h c", h=2)[:, :, 64:65], 1.0
            )

        # ---------------- emission helpers ----------------
        from concourse.tile import add_dep_helper

        first_mm = [None]  # tile-0 last matmul, for const-DMA deferral
        xt0_dma = [None]

        # all x tile loads issued upfront on sync (no deps -> no queue
        # blocking); dedicated tiles, 2KB/partition contiguous each.
        # xt0/xt1 go before the rope tables + vi so the first QKV matmul
        # isn't starved by the constant transfers.
        xts = [sb.tile([128, 8, 128], F16, tag=f"xt{i}", name=f"xt{i}") for i in range(NT)]
        xt_dmas = {}
        for i in range(4):
            xt_dmas[i] = nc.sync.dma_start(out=xts[i][:], in_=d_xtb[i])
        nc.sync.dma_start(out=cc_t[:], in_=d_cc[:])
        nc.sync.dma_start(out=sc_t[:], in_=d_sc[:])
        nc.sync.dma_start(out=vi_t[:], in_=d_vis[:])
        for i in range(4, NT):
            xt_dmas[i] = nc.sync.dma_start(out=xts[i][:], in_=d_xtb[i])
        xt0_dma[0] = xt_dmas[0]

        def emit_qkv_tile(ti):
            xt = xts[ti]
            psq = ps_q.tile([128, 384], F32, tag="psq", name=f"psq{ti}")
            for i in range(8):
                mm = nc.tensor.matmul(
                    psq[:], xt[:, i, :], wqkv[:, i, :],
                    start=(i == 0), stop=(i == 7),
                )
            if ti == 0:
                first_mm[0] = mm
                for cd in const_dmas:
                    add_dep_helper(cd.ins, mm.ins, True, "defer const DMA")
                for cd in early_dmas:
                    add_dep_helper(cd.ins, xt0_dma[0].ins, True, "defer early DMA")
            # v-blend straight from PSUM (DVE), fp16 out
            vt = vtiles[ti]
            nc.vector.scalar_tensor_tensor(
                out=vt[:].rearrange("p (s c) -> p s c", s=2)[:, :, 0:64],
                in0=psq[:, 256:384].rearrange("p (s c) -> p s c", s=2),
                scalar=1.0,
                in1=vi_t[:, ti, :].rearrange("p (s c) -> p s c", s=2),
                op0=ALU.mult, op1=ALU.add,
            )
            # stats: Square (ACT, reads PSUM) + segmented reduce (DVE)
            sqt = sb_w1.tile([128, 256], F16, tag="sqt", name=f"sqt{ti}")
            nc.scalar.activation(sqt[:], psq[:, 0:256], AF.Square)
            nc.vector.tensor_reduce(
                stats[:, 4 * ti : 4 * ti + 4],
                sqt[:].rearrange("p (s c) -> p s c", s=4),
                axis=AX.X, op=ALU.add,
            )
            # stage q,k to fp16 SBUF so the rope TTs run at DVE 2x; ACT
            # hosts the copy while it's exp-idle (opening tiles), DVE after
            sq16 = sb_w1.tile([128, 256], F16, tag="sq16", name=f"sq16_{ti}")
            if ti < 8:
                nc.scalar.copy(sq16[:], psq[:, 0:256])
            else:
                nc.vector.tensor_copy(sq16[:], psq[:, 0:256])
            # rope from the fp16 stage: swap absorbed into strided views;
            # tsin pair + tcos on DVE (all-fp16, 2x), add on Pool
            qk4 = sq16[:].rearrange("p (s h c) -> p s h c", s=4, h=2)
            tsin = sb_w2.tile([128, 256], F16, tag="tsin", name=f"tsin{ti}")
            t4 = tsin[:].rearrange("p (s h c) -> p s h c", s=4, h=2)
            nc.vector.tensor_tensor(
                out=t4[:, :, 0, :],
                in0=qk4[:, :, 1, :],
                in1=sc_t[:, ti, 0:32].unsqueeze(1).broadcast_to((128, 4, 32)),
                op=ALU.mult,
            )
            nc.vector.tensor_tensor(
                out=t4[:, :, 1, :],
                in0=qk4[:, :, 0, :],
                in1=sc_t[:, ti, 32:64].unsqueeze(1).broadcast_to((128, 4, 32)),
                op=ALU.mult,
            )
            tcos = sb_w2.tile([128, 256], F16, tag="tcos", name=f"tcos{ti}")
            nc.vector.tensor_tensor(
                out=tcos[:].rearrange("p (s c) -> p s c", s=4),
                in0=sq16[:].rearrange("p (s c) -> p s c", s=4),
                in1=cc_t[:, ti, :].unsqueeze(1).broadcast_to((128, 4, 64)),
                op=ALU.mult,
            )
            nc.gpsimd.tensor_tensor(
                out=qkro[ti][:], in0=tcos[:], in1=tsin[:], op=ALU.add
            )

        def emit_chain(g):
            # batched rsqrt for tiles 4g..4g+3 (DVE bit-trick + 1 Newton
            # iter), then norm applies + xbar DMA-transposes into qT/kT
            gg = 16 * g
            rs = rbuf[:, gg : gg + 16]
            zt = sb_w2.tile([128, 16], F32, tag="zt", name=f"zt{g}")
            nt1 = sb_w2.tile([128, 16], F32, tag="nt1", name=f"nt1{g}")
            nc.vector.tensor_scalar(
                out=zt[:], in0=stats[:, gg : gg + 16], scalar1=1.0 / 64.0,
                scalar2=RMS_EPS, op0=ALU.mult, op1=ALU.add,
            )
            nc.vector.tensor_scalar(
                out=nt1[:].bitcast(I32), in0=zt[:].bitcast(I32), scalar1=1,
                scalar2=0xFFFFFFFF, op0=ALU.logical_shift_right,
                op1=ALU.bitwise_xor,
            )
            nc.vector.tensor_scalar(
                out=rs.bitcast(I32), in0=nt1[:].bitcast(I32),
                scalar1=0x5F3759E0, scalar2=None, op0=ALU.add,
            )
            for _ in range(1):
                nc.vector.tensor_tensor(out=nt1[:], in0=rs, in1=rs, op=ALU.mult)
                nc.vector.tensor_tensor(out=nt1[:], in0=nt1[:], in1=zt[:], op=ALU.mult)
                nc.vector.tensor_scalar(
                    out=nt1[:], in0=nt1[:], scalar1=-0.5, scalar2=1.5,
                    op0=ALU.mult, op1=ALU.add,
                )
                nc.vector.tensor_tensor(out=rs, in0=rs, in1=nt1[:], op=ALU.mult)
            # fold 0.125 into the k columns of rbuf (cols 4t+2, 4t+3)
            kv = rbuf[:, gg : gg + 16].rearrange("p (t c) -> p t c", c=4)[:, :, 2:4]
            nc.vector.tensor_scalar_mul(kv, kv, 0.125)
            for tj in range(4 * g, 4 * g + 4):
                nc.vector.tensor_tensor(
                    out=qkr[tj][:].rearrange("p (s c) -> p s c", s=4),
                    in0=qkro[tj][:].rearrange("p (s c) -> p s c", s=4),
                    in1=rbuf[:, 4 * tj : 4 * tj + 4]
                    .unsqueeze(2)
                    .broadcast_to((128, 4, 64)),
                    op=ALU.mult,
                )
                for which, dst in ((0, qT), (1, kT)):
                    ptr = ps_q.tile(
                        [128, 128], F16, tag="psq", name=f"tr{tj}_{which}"
                    )
                    nc.tensor.transpose(
                        ptr[:], qkr[tj][:, 128 * which : 128 * which + 128], idn[:]
                    )
                    nc.vector.tensor_copy(
                        dst[:, 128 * tj : 128 * (tj + 1)], ptr[:]
                    )

        def emit_attention(ci, fillers, tail_fillers=()):
            # duo-units covering BOTH heads: 4 score matmuls (h0/h1 pairs
            # adjacent -> concurrent PE row-groups), exp per head, tri-mask
            # on diagonals, P@V lagging one unit.  Filler callbacks (QKV
            # tiles, chain, prev out-proj) are popped between units.
            kj_max = 4 * ci + 4
            yt_h = [
                ps_y.tile([128, 512], F32, tag="yt", name=f"yt{ci}_{h}")
                for h in range(2)
            ]
            units = list(range(kj_max // 2))
            ets = {}

            def emit_SE(dd):
                sts, etT = {}, {}
                qs = {}
                for j2 in range(2):
                    kj = 2 * dd + j2
                    qs[j2] = 128 * (kj - 4 * ci) if kj >= 4 * ci else 0
                for h in range(2):
                    sts[h] = ps_s.tile([128, 1024], F32, tag="st", name=f"st{ci}_{dd}_{h}")
                    etT[h] = sb_e.tile([128, 1024], F16, tag="et", name=f"et{ci}_{dd}_{h}")
                diag = [2 * dd + j2 >= 4 * ci for j2 in range(2)]
                for j2 in range(2):
                    kj = 2 * dd + j2
                    for h in range(2):
                        nc.tensor.matmul(
                            sts[h][:, 512 * j2 + qs[j2] : 512 * (j2 + 1)],
                            kT[64 * h : 64 * h + 64, 128 * kj : 128 * (kj + 1)],
                            qT[64 * h : 64 * h + 64, 512 * ci + qs[j2] : 512 * (ci + 1)],
                            start=True, stop=not diag[j2],
                        )
                # causal mask: accumulate -40 upper-triangle into the
                # diagonal block on the PE (keeps masking off DVE/Pool and
                # off the exp critical path); exp(s-40) ~ 0
                for j2 in range(2):
                    if diag[j2]:
                        for h in range(2):
                            nc.tensor.matmul(
                                sts[h][:, 512 * j2 + qs[j2] : 512 * j2 + qs[j2] + 128],
                                idn[:], tri[:],
                                start=False, stop=True,
                            )
                for h in range(2):
                    et = etT[h]
                    st = sts[h]
                    if qs[0] >= 256:
                        nc.scalar.activation(
                            et[:, qs[0] : 512], st[:, qs[0] : 512], AF.Exp
                        )
                        nc.scalar.activation(
                            et[:, 512 + qs[1] : 1024], st[:, 512 + qs[1] : 1024], AF.Exp
                        )
                    else:
                        nc.scalar.activation(et[:, qs[0] :], st[:, qs[0] :], AF.Exp)
                ets[dd] = (etT, qs)

            def emit_Y(dd):
                etT, qs = ets.pop(dd)
                for j2 in range(2):
                    kj = 2 * dd + j2
                    for h in range(2):
                        nc.tensor.matmul(
                            yt_h[h][0:65, qs[j2] : 512],
                            vtiles[kj][:, 65 * h : 65 * h + 65],
                            etT[h][:, 512 * j2 + qs[j2] : 512 * (j2 + 1)],
                            start=(kj == 0), stop=(kj == kj_max - 1 and j2 == 1),
                        )

            nf = len(fillers)
            n = len(units)
            for i, u in enumerate(units):
                # fillers first: they run on the PE while S/Y of this unit
                # wait on exp of the previous one (keeps HAM warm); pops are
                # front-loaded by one unit so the next chain's serial DVE
                # work starts before the window drains
                want = (nf * (i + 1)) // max(1, n - 1)
                while nf - len(fillers) < want and fillers:
                    fillers.pop(0)()
                emit_SE(u)
                if i >= 1:
                    emit_Y(units[i - 1])
            emit_Y(units[-1])
            for f in fillers:
                f()
            for f in tail_fillers:
                f()
            return yt_h

        def emit_scale_pieces(ci, yt_h):
            # returns filler callbacks computing scale + out-projection of
            # window ci (run interleaved into window ci+1's unit stream)
            lrs = [
                sb_m.tile([1, 512], F16, tag=f"lr{h}", name=f"lr{ci}_{h}")
                for h in range(2)
            ]
            mbs = sb_m.tile([128, 512], F32, tag="mbs", name=f"mbs{ci}")

            def piece_scale():
                # (L + e^sink)/16 in fp16, two K=1 head-indicator matmuls
                # accumulating a per-partition broadcast, reciprocal, then
                # scale with the 1/16 folded in
                mbp = ps_q.tile([128, 512], F32, tag="psq", name=f"mbp{ci}")
                for h in range(2):
                    nc.vector.scalar_tensor_tensor(
                        out=lrs[h][:],
                        in0=yt_h[h][64:65, 0:512],
                        scalar=0.0625,
                        in1=esk[0:1, h : h + 1].broadcast_to((1, 512)),
                        op0=ALU.mult, op1=ALU.add,
                    )
                    nc.tensor.matmul(
                        mbp[:], hind[0:1, 128 * h : 128 * (h + 1)], lrs[h][:],
                        start=(h == 0), stop=(h == 1),
                    )
                nc.vector.reciprocal_approx_fast(out=mbs[:], in_=mbp[:])

            def piece_apply(h):
                def f():
                    if h == 0:
                        nc.vector.scalar_tensor_tensor(
                            out=yts[0:64, 512 * ci : 512 * (ci + 1)],
                            in0=yt_h[h][0:64, 0:512],
                            scalar=0.0625,
                            in1=mbs[0:64, :],
                            op0=ALU.mult, op1=ALU.mult,
                        )
                    else:
                        yts1 = sb_w2.tile([64, 512], F16, tag="yts1", name=f"yts1_{ci}")
                        nc.vector.scalar_tensor_tensor(
                            out=yts1[:],
                            in0=yt_h[h][0:64, 0:512],
                            scalar=0.0625,
                            in1=mbs[64:128, :],
                            op0=ALU.mult, op1=ALU.mult,
                        )
                        nc.gpsimd.dma_start(
                            out=yts[64:128, 512 * ci : 512 * (ci + 1)], in_=yts1[:]
                        )
                return f

            outsb = [None]

            def piece_out(jt):
                def f():
                    if jt % 4 == 0:
                        outsb[0] = sb_o.tile(
                            [128, 4, 512], F16, tag="outsb", name=f"osb{ci}_{jt}"
                        )
                    pso = ps_q.tile([128, 512], F32, tag="psq", name=f"pso{ci}_{jt}")
                    nc.tensor.matmul(
                        pso[:],
                        wo[:, 128 * jt : 128 * (jt + 1)],
                        yts[:, 512 * ci : 512 * (ci + 1)],
                        start=True, stop=True,
                    )
                    if jt % 2 == 0:
                        nc.vector.tensor_copy(outsb[0][:, jt % 4, :], pso[:])
                    else:
                        nc.scalar.copy(outsb[0][:, jt % 4, :], pso[:])
                    if jt % 4 == 3:
                        # one batched DMA for 4 row-blocks (4KB/partition)
                        jt0 = jt - 3
                        nc.gpsimd.dma_start(
                            out=d_out[
                                128 * jt0 : 128 * (jt0 + 4),
                                512 * ci : 512 * (ci + 1),
                            ].rearrange("(c p) q -> p c q", c=4),
                            in_=outsb[0][:],
                        )
                return f

            return [piece_scale, piece_apply(0), piece_apply(1)] + [
                piece_out(jt) for jt in range(8)
            ]

        # ---------------- HAM warm-up: ~4us of junk matmuls ----------------
        pwz = ps_y.tile([128, 512], F32, tag="yt", name="pwz")
        for _w in range(6):
            nc.tensor.matmul(
                pwz[:], wz[:, 0:128], wz[:], start=True, stop=True
            )
        # preload the exp table set while the pipeline is still DMA-bound
        wze = sb_w2.tile([1, 2], F16, tag="wze", name="wze")
        nc.scalar.activation(wze[:], wz[0:1, 0:2], AF.Exp)

        # ---------------- interleaved emission ----------------
        # QKV 0-7 upfront (two chains) so the short early windows carry a
        # light DVE load; remaining tiles spread by window capacity.
        for ti in range(8):
            emit_qkv_tile(ti)
        emit_chain(0)
        emit_chain(1)
        qkv_per_window = {0: [8, 9], 1: [10, 11, 12, 13], 2: [14, 15], 3: []}
        chain_in_window = {1: 2, 2: 3}
        # out-projection of window w runs as filler in window outp_in[w]
        outp_in = {0: 2, 1: 3, 2: 3}
        yts_done = {}
        for ci in range(4):
            # QKV tiles + chain first (they feed the next window's critical
            # path on DVE), then deferred out-projections
            fillers = []
            for t in qkv_per_window[ci]:
                fillers.append(lambda t=t: emit_qkv_tile(t))
            if ci in chain_in_window:
                fillers.append(lambda g=chain_in_window[ci]: emit_chain(g))
            for w, host in outp_in.items():
                if host == ci:
                    fillers = fillers + emit_scale_pieces(w, yts_done[w])
            yts_done[ci] = emit_attention(ci, fillers)
        for f in emit_scale_pieces(3, yts_done[3]):
            f()

    nc.compile()
    return nc


_NC = None


def _rope_tables():
    inv = (1.0 / 10000.0) ** (np.arange(0, HD, 2, dtype=np.float64) / HD)
    t = np.arange(T, dtype=np.float64)
    f = np.outer(t, inv)  # (T, 32)
    cc = np.concatenate([np.cos(f), np.cos(f)], axis=1).astype(np.float32)
    sc = np.concatenate([np.sin(f), -np.sin(f)], axis=1).astype(np.float32)
    return cc, sc


def kernel(x, vi, Wq, Wk, Wv, Wo, lamb, sink_weights):
    global _NC
    x = np.asarray(x, dtype=np.float32)
    vi = np.asarray(vi, dtype=np.float32)
    Wq = np.asarray(Wq, dtype=np.float32)
    Wk = np.asarray(Wk, dtype=np.float32)
    Wv = np.asarray(Wv, dtype=np.float32)
    Wo = np.asarray(Wo, dtype=np.float32)
    lam = float(np.asarray(lamb).reshape(-1)[0])
    sink = np.asarray(sink_weights, dtype=np.float32).reshape(-1)

    if _NC is None:
        _NC = _build_program()

    x0T = x[0].T  # (D, T)
    xtb = np.ascontiguousarray(
        x0T.reshape(8, 128, NT, 128).transpose(2, 1, 0, 3)
    ).astype(np.float16)  # (NT, p, i, c): xtb[ti, p, n, c] = xT[128n+p, 128ti+c]
    cc, sc = _rope_tables()
    ccb = np.ascontiguousarray(cc.reshape(NT, 128, 64).transpose(1, 0, 2)).astype(
        np.float16
    )
    scb = np.ascontiguousarray(sc.reshape(NT, 128, 64).transpose(1, 0, 2)).astype(
        np.float16
    )
    # -40 strictly above the diagonal (key > query): exp(s-40) ~ 0
    tri = (-40.0 * (np.arange(128)[:, None] > np.arange(128)[None, :])).astype(
        np.float16
    )
    idn = np.eye(128, dtype=np.float16)
    hind = np.zeros((1, 256), np.float16)
    hind[0, 0:64] = 1.0
    hind[0, 192:256] = 1.0

    in_maps = []
    for c in range(8):
        lo = 128 * c
        wqkv = np.concatenate(
            [
                Wq[lo : lo + 128].T,
                Wk[lo : lo + 128].T,
                (1.0 - lam) * Wv[lo : lo + 128].T,
            ],
            axis=1,
        )  # (D, 384)
        wqkv = np.ascontiguousarray(
            wqkv.reshape(8, 128, 384).transpose(1, 0, 2)
        ).astype(np.float16)
        esk = (np.exp(sink[2 * c : 2 * c + 2]) / 16.0).astype(np.float16).reshape(1, 2)
        in_maps.append(
            {
                "xtb": xtb,
                "wqkv": wqkv,
                "vis": np.ascontiguousarray(
                    (lam * vi[0][:, lo : lo + 128]).reshape(NT, 128, 128).transpose(1, 0, 2)
                ).astype(np.float16),
                "cc": ccb,
                "sc": scb,
                "wo": np.ascontiguousarray(Wo[:, lo : lo + 128].T).astype(np.float16),
                "tri": tri,
                "idn": idn,
                "esk": esk,
                "hind": hind,
            }
        )

    global _trace_in_maps
    _trace_in_maps = in_maps
    res = None
    for attempt in range(3):
        try:
            res = run_bass_kernel_spmd(_NC, in_maps, list(range(8)))
            break
        except Exception:
            # transient NRT_EXEC_UNIT_UNRECOVERABLE flakes have been seen on
            # the first execute after a fresh compile; retry
            if attempt == 2:
                raise
    outT = np.zeros((D, T), np.float64)
    for c in range(8):
        outT += res.results[c]["out"].astype(np.float64)
    return np.ascontiguousarray(outT.T).astype(np.float32).reshape(1, T, D)


# revision 55
# speedup vs baseline: 1.0374x; 1.0374x over previous
"""Trainium2 Bass kernel for nn_CausalSelfAttention (B=1, T=2048, D=1024, H=16).

Sharding: 2 heads per core across 8 cores (tensor parallel). Wq/Wk/Wv
column-sharded by head, attention fully local, Wo row-sharded; host sums the
8 partial outputs (the all-reduce of the unshard step).

Per-core pipeline (all matmuls fp16 operands):
  P1  fused QKV: psq[t,384] = sum_i xT_blk.T @ [WqT|WkT|(1-l)WvT]; whole-tile
      x DMAs (2KB/partition, contiguous). RMS stats (Square on ACT from PSUM +
      segmented reduce -> rsqrt bit-trick, 1 Newton iter, 0.125 folded into
      k's scale), RoPE with the swap absorbed into strided views, v-blend
      (+lam*vi) via DVE scalar_tensor_tensor straight from PSUM.
  P2  qT/kT via xbar DMA-transpose (no PE, no DVE evac).
  P3  attention per duo-unit covering BOTH heads: 4 score matmuls into fp16
      PSUM with h0 (rows 0-63) / h1 (rows 64-127) pairs adjacent so the PE
      row-groups run concurrently; Exp (ACT) per head; tri-mask on diagonal
      blocks (DVE/Pool alternating); P@V lags one unit; lhsT=[v|1]
      accumulates L in row 64. QKV tiles for the next window, the previous
      window's out-projection, and the next chain/transposes are interleaved
      into the unit stream so the PE never idles (keeps HAM warm).
  P4  scale = 1/(L + e^sink): row-64 + e^sink -> K=2 head-indicator matmul
      broadcast -> reciprocal -> scale mult; out-proj per 128-row block into
      fp16 PSUM, fp16 evacuation alternating DVE/ACT, DMA out on sync/gpsimd.
"""

import sys

if "/opt/trn_rl_repo" not in sys.path:
    sys.path.insert(0, "/opt/trn_rl_repo")

import numpy as np
from contextlib import ExitStack

from concourse import bacc, tile
from concourse import mybir
from concourse.bass_utils import run_bass_kernel_spmd

F32 = mybir.dt.float32
F16 = mybir.dt.float16
I32 = mybir.dt.int32
AF = mybir.ActivationFunctionType
ALU = mybir.AluOpType
AX = mybir.AxisListType

T = 2048
D = 1024
HD = 64
NT = T // 128  # 16 t-tiles
RMS_EPS = float(np.finfo(np.float32).eps)


def _build_program():
    nc = bacc.Bacc("TRN2", target_bir_lowering=False, debug=False, num_devices=8)

    d_xtb = nc.dram_tensor("xtb", [NT, 128, 8, 128], F16, kind="ExternalInput").ap()
    d_wqkv = nc.dram_tensor("wqkv", [128, 8, 384], F16, kind="ExternalInput").ap()
    d_vis = nc.dram_tensor("vis", [128, NT, 128], F16, kind="ExternalInput").ap()
    d_cc = nc.dram_tensor("cc", [128, NT, 64], F16, kind="ExternalInput").ap()
    d_sc = nc.dram_tensor("sc", [128, NT, 64], F16, kind="ExternalInput").ap()
    d_wo = nc.dram_tensor("wo", [128, D], F16, kind="ExternalInput").ap()
    d_tri = nc.dram_tensor("tri", [128, 128], F16, kind="ExternalInput").ap()
    d_idn = nc.dram_tensor("idn", [128, 128], F16, kind="ExternalInput").ap()
    d_esk = nc.dram_tensor("esk", [1, 2], F16, kind="ExternalInput").ap()
    d_hind = nc.dram_tensor("hind", [1, 256], F16, kind="ExternalInput").ap()
    d_out = nc.dram_tensor("out", [D, T], F16, kind="ExternalOutput").ap()

    with tile.TileContext(nc) as tc, ExitStack() as ctx:
        sb = ctx.enter_context(tc.tile_pool(name="sb", bufs=1))
        sb_x = ctx.enter_context(tc.tile_pool(name="sb_x", bufs=4))
        sb_w1 = ctx.enter_context(tc.tile_pool(name="sb_w1", bufs=3))
        sb_w2 = ctx.enter_context(tc.tile_pool(name="sb_w2", bufs=3))
        sb_e = ctx.enter_context(tc.tile_pool(name="sb_e", bufs=8))
        sb_o = ctx.enter_context(tc.tile_pool(name="sb_o", bufs=3))
        sb_m = ctx.enter_context(tc.tile_pool(name="sb_m", bufs=3))
        # PSUM budget (8 banks): st scores f32 [128,1024] = 2 banks x 2 bufs;
        # ps_y the two per-window y accumulators; ps_q a shared 2-slot pool
        # for psq (QKV), pso/mbp (out-proj) and ptr (transposes).
        # 4 + 2 + 2 = 8 banks.
        ps_s = ctx.enter_context(tc.tile_pool(name="ps_s", bufs=2, space="PSUM"))
        ps_q = ctx.enter_context(tc.tile_pool(name="ps_q", bufs=2, space="PSUM"))
        ps_y = ctx.enter_context(tc.tile_pool(name="ps_y", bufs=2, space="PSUM"))

        # weights first on the sync queue (needed by the first matmul);
        # other constants go via the gpsimd queue so they don't delay x.
        wqkv = sb.tile([128, 8, 384], F16)
        wq_dmas = []
        for _wi in range(2):
            wq_dmas.append(
                nc.sync.dma_start(
                    out=wqkv[:, 4 * _wi : 4 * _wi + 4, :],
                    in_=d_wqkv[:, 4 * _wi : 4 * _wi + 4, :],
                )
            )
        vi_t = sb.tile([128, NT, 128], F16)
        cc_t = sb.tile([128, NT, 64], F16)
        sc_t = sb.tile([128, NT, 64], F16)
        wo = sb.tile([128, D], F16)
        esk = sb.tile([1, 2], F16)
        hind = sb.tile([1, 256], F16)
        const_dmas = []
        early_dmas = []
        early_dmas.append(nc.gpsimd.dma_start(out=esk[:], in_=d_esk[:]))
        tri = sb.tile([128, 128], F16)
        early_dmas.append(nc.gpsimd.dma_start(out=tri[:], in_=d_tri[:]))
        idn = sb.tile([128, 128], F16)
        early_dmas.append(nc.gpsimd.dma_start(out=idn[:], in_=d_idn[:]))
        const_dmas.append(nc.gpsimd.dma_start(out=wo[:], in_=d_wo[:]))
        const_dmas.append(nc.gpsimd.dma_start(out=hind[:], in_=d_hind[:]))

        stats = sb.tile([128, 64], F32)
        rbuf = sb.tile([128, 64], F32)
        qT = sb.tile([128, T], F16)
        kT = sb.tile([128, T], F16)
        vtiles = [sb.tile([128, 130], F16, tag=f"v{i}", name=f"v{i}") for i in range(NT)]
        qkr = [sb.tile([128, 256], F16, tag=f"qkr{i}", name=f"qkr{i}") for i in range(NT)]
        qkro = [sb.tile([128, 256], F16, tag=f"qkro{i}", name=f"qkro{i}") for i in range(NT)]
        yts = sb.tile([128, T], F16)

        # warm-up junk buffer first so the HAM warm-up matmuls start ASAP,
        # then the ones columns of the [vA|1|vB|1] tiles (written once)
        wz = sb.tile([128, 512], F16)
        nc.gpsimd.memset(wz[:], 0.0)
        for i in range(NT):
            nc.gpsimd.memset(
                vtiles[i][:].rearrange("p (h c) -> p h c", h=2)[:, :, 64:65], 1.0
            )

        # ---------------- emission helpers ----------------
        from concourse.tile import add_dep_helper

        first_mm = [None]  # tile-0 last matmul, for const-DMA deferral
        xt0_dma = [None]

        # all x tile loads issued upfront on sync (no deps -> no queue
        # blocking); dedicated tiles, 2KB/partition contiguous each.
        # xt0/xt1 go before the rope tables + vi so the first QKV matmul
        # isn't starved by the constant transfers.
        xts = [sb.tile([128, 8, 128], F16, tag=f"xt{i}", name=f"xt{i}") for i in range(NT)]
        xt_dmas = {}
        for i in (0, 1):
            xt_dmas[i] = nc.sync.dma_start(out=xts[i][:], in_=d_xtb[i])
        nc.sync.dma_start(out=cc_t[:], in_=d_cc[:])
        nc.sync.dma_start(out=sc_t[:], in_=d_sc[:])
        nc.sync.dma_start(out=vi_t[:], in_=d_vis[:])
        for i in range(2, NT):
            xt_dmas[i] = nc.sync.dma_start(out=xts[i][:], in_=d_xtb[i])
        xt0_dma[0] = xt_dmas[0]

        def emit_qkv_tile(ti):
            xt = xts[ti]
            psq = ps_q.tile([128, 384], F32, tag="psq", name=f"psq{ti}")
            for i in range(8):
                mm = nc.tensor.matmul(
                    psq[:], xt[:, i, :], wqkv[:, i, :],
                    start=(i == 0), stop=(i == 7),
                )
            if ti == 0:
                first_mm[0] = mm
                for cd in const_dmas:
                    add_dep_helper(cd.ins, mm.ins, True, "defer const DMA")
                for cd in early_dmas:
                    add_dep_helper(cd.ins, xt0_dma[0].ins, True, "defer early DMA")
            # v-blend straight from PSUM (DVE), fp16 out
            vt = vtiles[ti]
            nc.vector.scalar_tensor_tensor(
                out=vt[:].rearrange("p (s c) -> p s c", s=2)[:, :, 0:64],
                in0=psq[:, 256:384].rearrange("p (s c) -> p s c", s=2),
                scalar=1.0,
                in1=vi_t[:, ti, :].rearrange("p (s c) -> p s c", s=2),
                op0=ALU.mult, op1=ALU.add,
            )
            # stats: Square (ACT, reads PSUM) + segmented reduce (DVE)
            sqt = sb_w1.tile([128, 256], F16, tag="sqt", name=f"sqt{ti}")
            nc.scalar.activation(sqt[:], psq[:, 0:256], AF.Square)
            nc.vector.tensor_reduce(
                stats[:, 4 * ti : 4 * ti + 4],
                sqt[:].rearrange("p (s c) -> p s c", s=4),
                axis=AX.X, op=ALU.add,
            )
            # stage q,k to fp16 SBUF so the rope TTs run at DVE 2x; ACT
            # hosts the copy while it's exp-idle (opening tiles), DVE after
            sq16 = sb_w1.tile([128, 256], F16, tag="sq16", name=f"sq16_{ti}")
            nc.scalar.copy(sq16[:], psq[:, 0:256])
            # rope from the fp16 stage: swap absorbed into strided views;
            # tsin pair + tcos on DVE (all-fp16, 2x), add on Pool
            qk4 = sq16[:].rearrange("p (s h c) -> p s h c", s=4, h=2)
            tsin = sb_w2.tile([128, 256], F16, tag="tsin", name=f"tsin{ti}")
            t4 = tsin[:].rearrange("p (s h c) -> p s h c", s=4, h=2)
            nc.vector.tensor_tensor(
                out=t4[:, :, 0, :],
                in0=qk4[:, :, 1, :],
                in1=sc_t[:, ti, 0:32].unsqueeze(1).broadcast_to((128, 4, 32)),
                op=ALU.mult,
            )
            nc.vector.tensor_tensor(
                out=t4[:, :, 1, :],
                in0=qk4[:, :, 0, :],
                in1=sc_t[:, ti, 32:64].unsqueeze(1).broadcast_to((128, 4, 32)),
                op=ALU.mult,
            )
            tcos = sb_w2.tile([128, 256], F16, tag="tcos", name=f"tcos{ti}")
            nc.vector.tensor_tensor(
                out=tcos[:].rearrange("p (s c) -> p s c", s=4),
                in0=sq16[:].rearrange("p (s c) -> p s c", s=4),
                in1=cc_t[:, ti, :].unsqueeze(1).broadcast_to((128, 4, 64)),
                op=ALU.mult,
            )
            nc.gpsimd.tensor_tensor(
                out=qkro[ti][:], in0=tcos[:], in1=tsin[:], op=ALU.add
            )

        def emit_chain(g):
            # batched rsqrt for tiles 4g..4g+3 (DVE bit-trick + 1 Newton
            # iter), then norm applies + xbar DMA-transposes into qT/kT
            gg = 16 * g
            rs = rbuf[:, gg : gg + 16]
            zt = sb_w2.tile([128, 16], F32, tag="zt", name=f"zt{g}")
            nt1 = sb_w2.tile([128, 16], F32, tag="nt1", name=f"nt1{g}")
            nc.vector.tensor_scalar(
                out=zt[:], in0=stats[:, gg : gg + 16], scalar1=1.0 / 64.0,
                scalar2=RMS_EPS, op0=ALU.mult, op1=ALU.add,
            )
            nc.vector.tensor_scalar(
                out=nt1[:].bitcast(I32), in0=zt[:].bitcast(I32), scalar1=1,
                scalar2=0xFFFFFFFF, op0=ALU.logical_shift_right,
                op1=ALU.bitwise_xor,
            )
            nc.vector.tensor_scalar(
                out=rs.bitcast(I32), in0=nt1[:].bitcast(I32),
                scalar1=0x5F3759E0, scalar2=None, op0=ALU.add,
            )
            for _ in range(1):
                nc.vector.tensor_tensor(out=nt1[:], in0=rs, in1=rs, op=ALU.mult)
                nc.vector.tensor_tensor(out=nt1[:], in0=nt1[:], in1=zt[:], op=ALU.mult)
                nc.vector.tensor_scalar(
                    out=nt1[:], in0=nt1[:], scalar1=-0.5, scalar2=1.5,
                    op0=ALU.mult, op1=ALU.add,
                )
                nc.vector.tensor_tensor(out=rs, in0=rs, in1=nt1[:], op=ALU.mult)
            # fold 0.125 into the k columns of rbuf (cols 4t+2, 4t+3)
            kv = rbuf[:, gg : gg + 16].rearrange("p (t c) -> p t c", c=4)[:, :, 2:4]
            nc.vector.tensor_scalar_mul(kv, kv, 0.125)
            for tj in range(4 * g, 4 * g + 4):
                nc.vector.tensor_tensor(
                    out=qkr[tj][:].rearrange("p (s c) -> p s c", s=4),
                    in0=qkro[tj][:].rearrange("p (s c) -> p s c", s=4),
                    in1=rbuf[:, 4 * tj : 4 * tj + 4]
                    .unsqueeze(2)
                    .broadcast_to((128, 4, 64)),
                    op=ALU.mult,
                )
                for which, dst in ((0, qT), (1, kT)):
                    ptr = ps_q.tile(
                        [128, 128], F16, tag="psq", name=f"tr{tj}_{which}"
                    )
                    nc.tensor.transpose(
                        ptr[:], qkr[tj][:, 128 * which : 128 * which + 128], idn[:]
                    )
                    nc.vector.tensor_copy(
                        dst[:, 128 * tj : 128 * (tj + 1)], ptr[:]
                    )

        def emit_attention(ci, fillers, tail_fillers=()):
            # duo-units covering BOTH heads: 4 score matmuls (h0/h1 pairs
            # adjacent -> concurrent PE row-groups), exp per head, tri-mask
            # on diagonals, P@V lagging one unit.  Filler callbacks (QKV
            # tiles, chain, prev out-proj) are popped between units.
            kj_max = 4 * ci + 4
            yt_h = [
                ps_y.tile([128, 512], F32, tag="yt", name=f"yt{ci}_{h}")
                for h in range(2)
            ]
            units = list(range(kj_max // 2))
            ets = {}

            def emit_SE(dd):
                sts, etT = {}, {}
                qs = {}
                for j2 in range(2):
                    kj = 2 * dd + j2
                    qs[j2] = 128 * (kj - 4 * ci) if kj >= 4 * ci else 0
                for h in range(2):
                    sts[h] = ps_s.tile([128, 1024], F32, tag="st", name=f"st{ci}_{dd}_{h}")
                    etT[h] = sb_e.tile([128, 1024], F16, tag="et", name=f"et{ci}_{dd}_{h}")
                diag = [2 * dd + j2 >= 4 * ci for j2 in range(2)]
                for j2 in range(2):
                    kj = 2 * dd + j2
                    for h in range(2):
                        nc.tensor.matmul(
                            sts[h][:, 512 * j2 + qs[j2] : 512 * (j2 + 1)],
                            kT[64 * h : 64 * h + 64, 128 * kj : 128 * (kj + 1)],
                            qT[64 * h : 64 * h + 64, 512 * ci + qs[j2] : 512 * (ci + 1)],
                            start=True, stop=not diag[j2],
                        )
                # causal mask: accumulate -40 upper-triangle into the
                # diagonal block on the PE (keeps masking off DVE/Pool and
                # off the exp critical path); exp(s-40) ~ 0
                for j2 in range(2):
                    if diag[j2]:
                        for h in range(2):
                            nc.tensor.matmul(
                                sts[h][:, 512 * j2 + qs[j2] : 512 * j2 + qs[j2] + 128],
                                idn[:], tri[:],
                                start=False, stop=True,
                            )
                for h in range(2):
                    et = etT[h]
                    st = sts[h]
                    if qs[0] >= 256:
                        nc.scalar.activation(
                            et[:, qs[0] : 512], st[:, qs[0] : 512], AF.Exp
                        )
                        nc.scalar.activation(
                            et[:, 512 + qs[1] : 1024], st[:, 512 + qs[1] : 1024], AF.Exp
                        )
                    else:
                        nc.scalar.activation(et[:, qs[0] :], st[:, qs[0] :], AF.Exp)
                ets[dd] = (etT, qs)

            def emit_Y(dd):
                etT, qs = ets.pop(dd)
                for j2 in range(2):
                    kj = 2 * dd + j2
                    for h in range(2):
                        nc.tensor.matmul(
                            yt_h[h][0:65, qs[j2] : 512],
                            vtiles[kj][:, 65 * h : 65 * h + 65],
                            etT[h][:, 512 * j2 + qs[j2] : 512 * (j2 + 1)],
                            start=(kj == 0), stop=(kj == kj_max - 1 and j2 == 1),
                        )

            nf = len(fillers)
            n = len(units)
            for i, u in enumerate(units):
                # fillers first: they run on the PE while S/Y of this unit
                # wait on exp of the previous one (keeps HAM warm); pops are
                # front-loaded by one unit so the next chain's serial DVE
                # work starts before the window drains
                want = (nf * (i + 1)) // max(1, n - 1)
                while nf - len(fillers) < want and fillers:
                    fillers.pop(0)()
                emit_SE(u)
                # P@V lags TWO units behind (et is SBUF, 6 bufs): the Y
                # matmuls then never wait on exp
                if i >= 2:
                    emit_Y(units[i - 2])
            if len(units) >= 2:
                emit_Y(units[-2])
            emit_Y(units[-1])
            for f in fillers:
                f()
            for f in tail_fillers:
                f()
            return yt_h

        def emit_scale_pieces(ci, yt_h):
            # returns filler callbacks computing scale + out-projection of
            # window ci (run interleaved into window ci+1's unit stream)
            lrs = [
                sb_m.tile([1, 512], F16, tag=f"lr{h}", name=f"lr{ci}_{h}")
                for h in range(2)
            ]
            mbs = sb_m.tile([128, 512], F32, tag="mbs", name=f"mbs{ci}")

            def piece_scale():
                # (L + e^sink)/16 in fp16, two K=1 head-indicator matmuls
                # accumulating a per-partition broadcast, reciprocal, then
                # scale with the 1/16 folded in
                mbp = ps_q.tile([128, 512], F32, tag="psq", name=f"mbp{ci}")
                for h in range(2):
                    nc.vector.scalar_tensor_tensor(
                        out=lrs[h][:],
                        in0=yt_h[h][64:65, 0:512],
                        scalar=0.0625,
                        in1=esk[0:1, h : h + 1].broadcast_to((1, 512)),
                        op0=ALU.mult, op1=ALU.add,
                    )
                    nc.tensor.matmul(
                        mbp[:], hind[0:1, 128 * h : 128 * (h + 1)], lrs[h][:],
                        start=(h == 0), stop=(h == 1),
                    )
                nc.vector.reciprocal_approx_fast(out=mbs[:], in_=mbp[:])

            def piece_apply(h):
                def f():
                    if h == 0:
                        nc.vector.scalar_tensor_tensor(
                            out=yts[0:64, 512 * ci : 512 * (ci + 1)],
                            in0=yt_h[h][0:64, 0:512],
                            scalar=0.0625,
                            in1=mbs[0:64, :],
                            op0=ALU.mult, op1=ALU.mult,
                        )
                    else:
                        yts1 = sb_w2.tile([64, 512], F16, tag="yts1", name=f"yts1_{ci}")
                        nc.vector.scalar_tensor_tensor(
                            out=yts1[:],
                            in0=yt_h[h][0:64, 0:512],
                            scalar=0.0625,
                            in1=mbs[64:128, :],
                            op0=ALU.mult, op1=ALU.mult,
                        )
                        nc.gpsimd.dma_start(
                            out=yts[64:128, 512 * ci : 512 * (ci + 1)], in_=yts1[:]
                        )
                return f

            outsb = [None]

            def piece_out(jt):
                def f():
                    if jt % 4 == 0:
                        outsb[0] = sb_o.tile(
                            [128, 4, 512], F16, tag="outsb", name=f"osb{ci}_{jt}"
                        )
                    pso = ps_q.tile([128, 512], F32, tag="psq", name=f"pso{ci}_{jt}")
                    nc.tensor.matmul(
                        pso[:],
                        wo[:, 128 * jt : 128 * (jt + 1)],
                        yts[:, 512 * ci : 512 * (ci + 1)],
                        start=True, stop=True,
                    )
                    if jt % 2 == 0:
                        nc.vector.tensor_copy(outsb[0][:, jt % 4, :], pso[:])
                    else:
                        nc.scalar.copy(outsb[0][:, jt % 4, :], pso[:])
                    if jt % 4 == 3:
                        # one batched DMA for 4 row-blocks (4KB/partition)
                        jt0 = jt - 3
                        nc.gpsimd.dma_start(
                            out=d_out[
                                128 * jt0 : 128 * (jt0 + 4),
                                512 * ci : 512 * (ci + 1),
                            ].rearrange("(c p) q -> p c q", c=4),
                            in_=outsb[0][:],
                        )
                return f

            return [piece_scale, piece_apply(0), piece_apply(1)] + [
                piece_out(jt) for jt in range(8)
            ]

        # ---------------- HAM warm-up: ~4us of junk matmuls ----------------
        pwz = ps_y.tile([128, 512], F32, tag="yt", name="pwz")
        for _w in range(6):
            nc.tensor.matmul(
                pwz[:], wz[:, 0:128], wz[:], start=True, stop=True
            )
        # preload the exp table set while the pipeline is still DMA-bound
        wze = sb_w2.tile([1, 2], F16, tag="wze", name="wze")
        nc.scalar.activation(wze[:], wz[0:1, 0:2], AF.Exp)

        # ---------------- interleaved emission ----------------
        # QKV 0-7 upfront (two chains) so the short early windows carry a
        # light DVE load; remaining tiles spread by window capacity.
        for ti in range(8):
            emit_qkv_tile(ti)
        emit_chain(0)
        emit_chain(1)
        qkv_per_window = {0: [8, 9], 1: [10, 11], 2: [12, 13, 14, 15], 3: []}
        chain_in_window = {1: 2, 2: 3}
        # out-projection of window w runs as filler in window outp_in[w]
        # (w2's DVE is already full with QKV 12-15 + chain 3; w3 is thin)
        outp_in = {0: 1, 1: 3, 2: 3}
        yts_done = {}
        for ci in range(4):
            # QKV tiles + chain first (they feed the next window's critical
            # path on DVE), then deferred out-projections
            fillers = []
            for t in qkv_per_window[ci]:
                fillers.append(lambda t=t: emit_qkv_tile(t))
            if ci in chain_in_window:
                fillers.append(lambda g=chain_in_window[ci]: emit_chain(g))
            for w, host in outp_in.items():
                if host == ci:
                    fillers = fillers + emit_scale_pieces(w, yts_done[w])
            yts_done[ci] = emit_attention(ci, fillers)
        for f in emit_scale_pieces(3, yts_done[3]):
            f()

    nc.compile()
    return nc


_NC = None


def _rope_tables():
    inv = (1.0 / 10000.0) ** (np.arange(0, HD, 2, dtype=np.float64) / HD)
    t = np.arange(T, dtype=np.float64)
    f = np.outer(t, inv)  # (T, 32)
    cc = np.concatenate([np.cos(f), np.cos(f)], axis=1).astype(np.float32)
    sc = np.concatenate([np.sin(f), -np.sin(f)], axis=1).astype(np.float32)
    return cc, sc


def kernel(x, vi, Wq, Wk, Wv, Wo, lamb, sink_weights):
    global _NC
    x = np.asarray(x, dtype=np.float32)
    vi = np.asarray(vi, dtype=np.float32)
    Wq = np.asarray(Wq, dtype=np.float32)
    Wk = np.asarray(Wk, dtype=np.float32)
    Wv = np.asarray(Wv, dtype=np.float32)
    Wo = np.asarray(Wo, dtype=np.float32)
    lam = float(np.asarray(lamb).reshape(-1)[0])
    sink = np.asarray(sink_weights, dtype=np.float32).reshape(-1)

    if _NC is None:
        _NC = _build_program()

    x0T = x[0].T  # (D, T)
    xtb = np.ascontiguousarray(
        x0T.reshape(8, 128, NT, 128).transpose(2, 1, 0, 3)
    ).astype(np.float16)  # (NT, p, i, c): xtb[ti, p, n, c] = xT[128n+p, 128ti+c]
    cc, sc = _rope_tables()
    ccb = np.ascontiguousarray(cc.reshape(NT, 128, 64).transpose(1, 0, 2)).astype(
        np.float16
    )
    scb = np.ascontiguousarray(sc.reshape(NT, 128, 64).transpose(1, 0, 2)).astype(
        np.float16
    )
    # -40 strictly above the diagonal (key > query): exp(s-40) ~ 0
    tri = (-40.0 * (np.arange(128)[:, None] > np.arange(128)[None, :])).astype(
        np.float16
    )
    idn = np.eye(128, dtype=np.float16)
    hind = np.zeros((1, 256), np.float16)
    hind[0, 0:64] = 1.0
    hind[0, 192:256] = 1.0

    in_maps = []
    for c in range(8):
        lo = 128 * c
        wqkv = np.concatenate(
            [
                Wq[lo : lo + 128].T,
                Wk[lo : lo + 128].T,
                (1.0 - lam) * Wv[lo : lo + 128].T,
            ],
            axis=1,
        )  # (D, 384)
        wqkv = np.ascontiguousarray(
            wqkv.reshape(8, 128, 384).transpose(1, 0, 2)
        ).astype(np.float16)
        esk = (np.exp(sink[2 * c : 2 * c + 2]) / 16.0).astype(np.float16).reshape(1, 2)
        in_maps.append(
            {
                "xtb": xtb,
                "wqkv": wqkv,
                "vis": np.ascontiguousarray(
                    (lam * vi[0][:, lo : lo + 128]).reshape(NT, 128, 128).transpose(1, 0, 2)
                ).astype(np.float16),
                "cc": ccb,
                "sc": scb,
                "wo": np.ascontiguousarray(Wo[:, lo : lo + 128].T).astype(np.float16),
                "tri": tri,
                "idn": idn,
                "esk": esk,
                "hind": hind,
            }
        )

    global _trace_in_maps
    _trace_in_maps = in_maps
    res = None
    for attempt in range(3):
        try:
            res = run_bass_kernel_spmd(_NC, in_maps, list(range(8)))
            break
        except Exception:
            # transient NRT_EXEC_UNIT_UNRECOVERABLE flakes have been seen on
            # the first execute after a fresh compile; retry
            if attempt == 2:
                raise
    outT = np.zeros((D, T), np.float64)
    for c in range(8):
        outT += res.results[c]["out"].astype(np.float64)
    return np.ascontiguousarray(outT.T).astype(np.float32).reshape(1, T, D)


# revision 58
# speedup vs baseline: 1.0795x; 1.0406x over previous
"""Trainium2 Bass kernel for nn_CausalSelfAttention (B=1, T=2048, D=1024, H=16).

Sharding: 2 heads per core across 8 cores (tensor parallel). Wq/Wk/Wv
column-sharded by head, attention fully local, Wo row-sharded; host sums the
8 partial outputs (the all-reduce of the unshard step).

Per-core pipeline (all matmuls fp16 operands):
  P1  fused QKV: psq[t,384] = sum_i xT_blk.T @ [WqT|WkT|(1-l)WvT]; whole-tile
      x DMAs (2KB/partition, contiguous). RMS stats (Square on ACT from PSUM +
      segmented reduce -> rsqrt bit-trick, 1 Newton iter, 0.125 folded into
      k's scale), RoPE with the swap absorbed into strided views, v-blend
      (+lam*vi) via DVE scalar_tensor_tensor straight from PSUM.
  P2  qT/kT via xbar DMA-transpose (no PE, no DVE evac).
  P3  attention per duo-unit covering BOTH heads: 4 score matmuls into fp16
      PSUM with h0 (rows 0-63) / h1 (rows 64-127) pairs adjacent so the PE
      row-groups run concurrently; Exp (ACT) per head; tri-mask on diagonal
      blocks (DVE/Pool alternating); P@V lags one unit; lhsT=[v|1]
      accumulates L in row 64. QKV tiles for the next window, the previous
      window's out-projection, and the next chain/transposes are interleaved
      into the unit stream so the PE never idles (keeps HAM warm).
  P4  scale = 1/(L + e^sink): row-64 + e^sink -> K=2 head-indicator matmul
      broadcast -> reciprocal -> scale mult; out-proj per 128-row block into
      fp16 PSUM, fp16 evacuation alternating DVE/ACT, DMA out on sync/gpsimd.
"""

import sys

if "/opt/trn_rl_repo" not in sys.path:
    sys.path.insert(0, "/opt/trn_rl_repo")

import numpy as np
from contextlib import ExitStack

from concourse import bacc, tile
from concourse import mybir
from concourse.bass_utils import run_bass_kernel_spmd

F32 = mybir.dt.float32
F16 = mybir.dt.float16
I32 = mybir.dt.int32
AF = mybir.ActivationFunctionType
ALU = mybir.AluOpType
AX = mybir.AxisListType

T = 2048
D = 1024
HD = 64
NT = T // 128  # 16 t-tiles
RMS_EPS = float(np.finfo(np.float32).eps)


def _build_program():
    nc = bacc.Bacc("TRN2", target_bir_lowering=False, debug=False, num_devices=8)

    d_xtb = nc.dram_tensor("xtb", [NT, 128, 8, 128], F16, kind="ExternalInput").ap()
    d_wqkv = nc.dram_tensor("wqkv", [128, 8, 384], F16, kind="ExternalInput").ap()
    d_vis = nc.dram_tensor("vis", [128, NT, 128], F16, kind="ExternalInput").ap()
    d_cc = nc.dram_tensor("cc", [128, NT, 64], F16, kind="ExternalInput").ap()
    d_sc = nc.dram_tensor("sc", [128, NT, 64], F16, kind="ExternalInput").ap()
    d_wo = nc.dram_tensor("wo", [128, D], F16, kind="ExternalInput").ap()
    d_tri = nc.dram_tensor("tri", [128, 128], F16, kind="ExternalInput").ap()
    d_idn = nc.dram_tensor("idn", [128, 128], F16, kind="ExternalInput").ap()
    d_esk = nc.dram_tensor("esk", [1, 2], F16, kind="ExternalInput").ap()
    d_hind = nc.dram_tensor("hind", [1, 256], F16, kind="ExternalInput").ap()
    d_out = nc.dram_tensor("out", [D, T], F16, kind="ExternalOutput").ap()

    with tile.TileContext(nc) as tc, ExitStack() as ctx:
        sb = ctx.enter_context(tc.tile_pool(name="sb", bufs=1))
        sb_x = ctx.enter_context(tc.tile_pool(name="sb_x", bufs=4))
        sb_w1 = ctx.enter_context(tc.tile_pool(name="sb_w1", bufs=3))
        sb_w2 = ctx.enter_context(tc.tile_pool(name="sb_w2", bufs=3))
        sb_e = ctx.enter_context(tc.tile_pool(name="sb_e", bufs=8))
        sb_o = ctx.enter_context(tc.tile_pool(name="sb_o", bufs=3))
        sb_m = ctx.enter_context(tc.tile_pool(name="sb_m", bufs=3))
        # PSUM budget (8 banks): st scores f32 [128,1024] = 2 banks x 2 bufs;
        # ps_y the two per-window y accumulators; ps_q a shared 2-slot pool
        # for psq (QKV), pso/mbp (out-proj) and ptr (transposes).
        # 4 + 2 + 2 = 8 banks.
        ps_s = ctx.enter_context(tc.tile_pool(name="ps_s", bufs=2, space="PSUM"))
        ps_q = ctx.enter_context(tc.tile_pool(name="ps_q", bufs=2, space="PSUM"))
        ps_y = ctx.enter_context(tc.tile_pool(name="ps_y", bufs=2, space="PSUM"))

        # weights first on the sync queue (needed by the first matmul);
        # other constants go via the gpsimd queue so they don't delay x.
        wqkv = sb.tile([128, 8, 384], F16)
        wq_dmas = []
        for _wi in range(2):
            wq_dmas.append(
                nc.sync.dma_start(
                    out=wqkv[:, 4 * _wi : 4 * _wi + 4, :],
                    in_=d_wqkv[:, 4 * _wi : 4 * _wi + 4, :],
                )
            )
        vi_t = sb.tile([128, NT, 128], F16)
        cc_t = sb.tile([128, NT, 64], F16)
        sc_t = sb.tile([128, NT, 64], F16)
        wo = sb.tile([128, D], F16)
        esk = sb.tile([1, 2], F16)
        hind = sb.tile([1, 256], F16)
        const_dmas = []
        early_dmas = []
        early_dmas.append(nc.gpsimd.dma_start(out=esk[:], in_=d_esk[:]))
        tri = sb.tile([128, 128], F16)
        early_dmas.append(nc.gpsimd.dma_start(out=tri[:], in_=d_tri[:]))
        idn = sb.tile([128, 128], F16)
        early_dmas.append(nc.gpsimd.dma_start(out=idn[:], in_=d_idn[:]))
        const_dmas.append(nc.gpsimd.dma_start(out=wo[:], in_=d_wo[:]))
        const_dmas.append(nc.gpsimd.dma_start(out=hind[:], in_=d_hind[:]))

        stats = sb.tile([128, 64], F32)
        rbuf = sb.tile([128, 64], F32)
        qT = sb.tile([128, T], F16)
        kT = sb.tile([128, T], F16)
        vtiles = [sb.tile([128, 130], F16, tag=f"v{i}", name=f"v{i}") for i in range(NT)]
        qkr = [sb.tile([128, 256], F16, tag=f"qkr{i}", name=f"qkr{i}") for i in range(NT)]
        qkro = [sb.tile([128, 256], F16, tag=f"qkro{i}", name=f"qkro{i}") for i in range(NT)]
        yts = sb.tile([128, T], F16)

        # warm-up junk buffer first so the HAM warm-up matmuls start ASAP,
        # then the ones columns of the [vA|1|vB|1] tiles (written once)
        wz = sb.tile([128, 512], F16)
        nc.gpsimd.memset(wz[:], 0.0)
        for i in range(NT):
            nc.gpsimd.memset(
                vtiles[i][:].rearrange("p (h c) -> p h c", h=2)[:, :, 64:65], 1.0
            )

        # ---------------- emission helpers ----------------
        from concourse.tile import add_dep_helper

        first_mm = [None]  # tile-0 last matmul, for const-DMA deferral
        xt0_dma = [None]

        # all x tile loads issued upfront on sync (no deps -> no queue
        # blocking); dedicated tiles, 2KB/partition contiguous each.
        # xt0/xt1 go before the rope tables + vi so the first QKV matmul
        # isn't starved by the constant transfers.
        xts = [sb.tile([128, 8, 128], F16, tag=f"xt{i}", name=f"xt{i}") for i in range(NT)]
        xt_dmas = {}
        for i in (0, 1):
            xt_dmas[i] = nc.sync.dma_start(out=xts[i][:], in_=d_xtb[i])
        nc.sync.dma_start(out=cc_t[:], in_=d_cc[:])
        nc.sync.dma_start(out=sc_t[:], in_=d_sc[:])
        nc.sync.dma_start(out=vi_t[:], in_=d_vis[:])
        for i in range(2, NT):
            xt_dmas[i] = nc.sync.dma_start(out=xts[i][:], in_=d_xtb[i])
        xt0_dma[0] = xt_dmas[0]

        def emit_qkv_tile(ti):
            xt = xts[ti]
            psq = ps_q.tile([128, 384], F32, tag="psq", name=f"psq{ti}")
            for i in range(8):
                mm = nc.tensor.matmul(
                    psq[:], xt[:, i, :], wqkv[:, i, :],
                    start=(i == 0), stop=(i == 7),
                )
            if ti == 0:
                first_mm[0] = mm
                for cd in const_dmas:
                    add_dep_helper(cd.ins, mm.ins, True, "defer const DMA")
                for cd in early_dmas:
                    add_dep_helper(cd.ins, xt0_dma[0].ins, True, "defer early DMA")
            # v-blend straight from PSUM (DVE), fp16 out
            vt = vtiles[ti]
            nc.vector.scalar_tensor_tensor(
                out=vt[:].rearrange("p (s c) -> p s c", s=2)[:, :, 0:64],
                in0=psq[:, 256:384].rearrange("p (s c) -> p s c", s=2),
                scalar=1.0,
                in1=vi_t[:, ti, :].rearrange("p (s c) -> p s c", s=2),
                op0=ALU.mult, op1=ALU.add,
            )
            # stats: Square (ACT, reads PSUM) + segmented reduce (DVE)
            sqt = sb_w1.tile([128, 256], F16, tag="sqt", name=f"sqt{ti}")
            nc.scalar.activation(sqt[:], psq[:, 0:256], AF.Square)
            nc.vector.tensor_reduce(
                stats[:, 4 * ti : 4 * ti + 4],
                sqt[:].rearrange("p (s c) -> p s c", s=4),
                axis=AX.X, op=ALU.add,
            )
            # stage q,k to fp16 SBUF so the rope TTs run at DVE 2x; ACT
            # hosts the copy while it's exp-idle (opening tiles), DVE after
            sq16 = sb_w1.tile([128, 256], F16, tag="sq16", name=f"sq16_{ti}")
            nc.scalar.copy(sq16[:], psq[:, 0:256])
            # rope from the fp16 stage: swap absorbed into strided views;
            # tsin pair + tcos on DVE (all-fp16, 2x), add on Pool
            qk4 = sq16[:].rearrange("p (s h c) -> p s h c", s=4, h=2)
            tsin = sb_w2.tile([128, 256], F16, tag="tsin", name=f"tsin{ti}")
            t4 = tsin[:].rearrange("p (s h c) -> p s h c", s=4, h=2)
            nc.vector.tensor_tensor(
                out=t4[:, :, 0, :],
                in0=qk4[:, :, 1, :],
                in1=sc_t[:, ti, 0:32].unsqueeze(1).broadcast_to((128, 4, 32)),
                op=ALU.mult,
            )
            nc.vector.tensor_tensor(
                out=t4[:, :, 1, :],
                in0=qk4[:, :, 0, :],
                in1=sc_t[:, ti, 32:64].unsqueeze(1).broadcast_to((128, 4, 32)),
                op=ALU.mult,
            )
            tcos = sb_w2.tile([128, 256], F16, tag="tcos", name=f"tcos{ti}")
            nc.vector.tensor_tensor(
                out=tcos[:].rearrange("p (s c) -> p s c", s=4),
                in0=sq16[:].rearrange("p (s c) -> p s c", s=4),
                in1=cc_t[:, ti, :].unsqueeze(1).broadcast_to((128, 4, 64)),
                op=ALU.mult,
            )
            nc.gpsimd.tensor_tensor(
                out=qkro[ti][:], in0=tcos[:], in1=tsin[:], op=ALU.add
            )

        def emit_chain(g):
            # batched rsqrt for tiles 4g..4g+3 (DVE bit-trick + 1 Newton
            # iter), then norm applies + xbar DMA-transposes into qT/kT
            gg = 16 * g
            rs = rbuf[:, gg : gg + 16]
            zt = sb_w2.tile([128, 16], F32, tag="zt", name=f"zt{g}")
            nt1 = sb_w2.tile([128, 16], F32, tag="nt1", name=f"nt1{g}")
            nc.vector.tensor_scalar(
                out=zt[:], in0=stats[:, gg : gg + 16], scalar1=1.0 / 64.0,
                scalar2=RMS_EPS, op0=ALU.mult, op1=ALU.add,
            )
            nc.vector.tensor_scalar(
                out=nt1[:].bitcast(I32), in0=zt[:].bitcast(I32), scalar1=1,
                scalar2=0xFFFFFFFF, op0=ALU.logical_shift_right,
                op1=ALU.bitwise_xor,
            )
            nc.vector.tensor_scalar(
                out=rs.bitcast(I32), in0=nt1[:].bitcast(I32),
                scalar1=0x5F3759E0, scalar2=None, op0=ALU.add,
            )
            for _ in range(1):
                nc.vector.tensor_tensor(out=nt1[:], in0=rs, in1=rs, op=ALU.mult)
                nc.vector.tensor_tensor(out=nt1[:], in0=nt1[:], in1=zt[:], op=ALU.mult)
                nc.vector.tensor_scalar(
                    out=nt1[:], in0=nt1[:], scalar1=-0.5, scalar2=1.5,
                    op0=ALU.mult, op1=ALU.add,
                )
                nc.vector.tensor_tensor(out=rs, in0=rs, in1=nt1[:], op=ALU.mult)
            # fold 0.125 into the k columns of rbuf (cols 4t+2, 4t+3)
            kv = rbuf[:, gg : gg + 16].rearrange("p (t c) -> p t c", c=4)[:, :, 2:4]
            nc.vector.tensor_scalar_mul(kv, kv, 0.125)
            for tj in range(4 * g, 4 * g + 4):
                nc.vector.tensor_tensor(
                    out=qkr[tj][:].rearrange("p (s c) -> p s c", s=4),
                    in0=qkro[tj][:].rearrange("p (s c) -> p s c", s=4),
                    in1=rbuf[:, 4 * tj : 4 * tj + 4]
                    .unsqueeze(2)
                    .broadcast_to((128, 4, 64)),
                    op=ALU.mult,
                )
                for which, dst in ((0, qT), (1, kT)):
                    ptr = ps_q.tile(
                        [128, 128], F16, tag="psq", name=f"tr{tj}_{which}"
                    )
                    nc.tensor.transpose(
                        ptr[:], qkr[tj][:, 128 * which : 128 * which + 128], idn[:]
                    )
                    nc.vector.tensor_copy(
                        dst[:, 128 * tj : 128 * (tj + 1)], ptr[:]
                    )

        def emit_attention(ci, fillers, tail_fillers=()):
            # duo-units covering BOTH heads: 4 score matmuls (h0/h1 pairs
            # adjacent -> concurrent PE row-groups), exp per head, tri-mask
            # on diagonals, P@V lagging one unit.  Filler callbacks (QKV
            # tiles, chain, prev out-proj) are popped between units.
            kj_max = 4 * ci + 4
            yt_h = [
                ps_y.tile([128, 512], F32, tag="yt", name=f"yt{ci}_{h}")
                for h in range(2)
            ]
            units = list(range(kj_max // 2))
            ets = {}

            def emit_SE(dd):
                sts, etT = {}, {}
                qs = {}
                for j2 in range(2):
                    kj = 2 * dd + j2
                    qs[j2] = 128 * (kj - 4 * ci) if kj >= 4 * ci else 0
                for h in range(2):
                    sts[h] = ps_s.tile([128, 1024], F32, tag="st", name=f"st{ci}_{dd}_{h}")
                    etT[h] = sb_e.tile([128, 1024], F16, tag="et", name=f"et{ci}_{dd}_{h}")
                diag = [2 * dd + j2 >= 4 * ci for j2 in range(2)]
                # compacted layout: block j2=0 lives at [qs0:512], block
                # j2=1 at [512 : 1024-qs1] -- one contiguous live region so
                # each head's exp is a single instruction with no dead cols
                base = (0, qs[1])
                for j2 in range(2):
                    kj = 2 * dd + j2
                    for h in range(2):
                        nc.tensor.matmul(
                            sts[h][:, 512 * j2 + qs[j2] - base[j2] : 512 * (j2 + 1) - base[j2]],
                            kT[64 * h : 64 * h + 64, 128 * kj : 128 * (kj + 1)],
                            qT[64 * h : 64 * h + 64, 512 * ci + qs[j2] : 512 * (ci + 1)],
                            start=True, stop=not diag[j2],
                        )
                # causal mask: accumulate -40 upper-triangle into the
                # diagonal block on the PE (keeps masking off DVE/Pool and
                # off the exp critical path); exp(s-40) ~ 0
                for j2 in range(2):
                    if diag[j2]:
                        for h in range(2):
                            b = 512 * j2 + qs[j2] - base[j2]
                            nc.tensor.matmul(
                                sts[h][:, b : b + 128],
                                idn[:], tri[:],
                                start=False, stop=True,
                            )
                for h in range(2):
                    nc.scalar.activation(
                        etT[h][:, qs[0] : 1024 - base[1]],
                        sts[h][:, qs[0] : 1024 - base[1]],
                        AF.Exp,
                    )
                ets[dd] = (etT, qs, base)

            def emit_Y(dd):
                etT, qs, base = ets.pop(dd)
                for j2 in range(2):
                    kj = 2 * dd + j2
                    for h in range(2):
                        nc.tensor.matmul(
                            yt_h[h][0:65, qs[j2] : 512],
                            vtiles[kj][:, 65 * h : 65 * h + 65],
                            etT[h][:, 512 * j2 + qs[j2] - base[j2] : 512 * (j2 + 1) - base[j2]],
                            start=(kj == 0), stop=(kj == kj_max - 1 and j2 == 1),
                        )

            nf = len(fillers)
            n = len(units)
            for i, u in enumerate(units):
                # fillers first: they run on the PE while S/Y of this unit
                # wait on exp of the previous one (keeps HAM warm); pops are
                # front-loaded by one unit so the next chain's serial DVE
                # work starts before the window drains
                want = (nf * (i + 1)) // max(1, n - 1)
                while nf - len(fillers) < want and fillers:
                    fillers.pop(0)()
                emit_SE(u)
                # P@V lags TWO units behind (et is SBUF, 6 bufs): the Y
                # matmuls then never wait on exp
                if i >= 2:
                    emit_Y(units[i - 2])
            if len(units) >= 2:
                emit_Y(units[-2])
            emit_Y(units[-1])
            for f in fillers:
                f()
            for f in tail_fillers:
                f()
            return yt_h

        def emit_scale_pieces(ci, yt_h):
            # returns filler callbacks computing scale + out-projection of
            # window ci (run interleaved into window ci+1's unit stream)
            lrs = [
                sb_m.tile([1, 512], F16, tag=f"lr{h}", name=f"lr{ci}_{h}")
                for h in range(2)
            ]
            mbs = sb_m.tile([128, 512], F32, tag="mbs", name=f"mbs{ci}")

            def piece_scale():
                # (L + e^sink)/16 in fp16, two K=1 head-indicator matmuls
                # accumulating a per-partition broadcast, reciprocal, then
                # scale with the 1/16 folded in
                mbp = ps_q.tile([128, 512], F32, tag="psq", name=f"mbp{ci}")
                for h in range(2):
                    nc.vector.scalar_tensor_tensor(
                        out=lrs[h][:],
                        in0=yt_h[h][64:65, 0:512],
                        scalar=0.0625,
                        in1=esk[0:1, h : h + 1].broadcast_to((1, 512)),
                        op0=ALU.mult, op1=ALU.add,
                    )
                    nc.tensor.matmul(
                        mbp[:], hind[0:1, 128 * h : 128 * (h + 1)], lrs[h][:],
                        start=(h == 0), stop=(h == 1),
                    )
                nc.vector.reciprocal_approx_fast(out=mbs[:], in_=mbp[:])

            def piece_apply(h):
                def f():
                    if h == 0:
                        nc.vector.scalar_tensor_tensor(
                            out=yts[0:64, 512 * ci : 512 * (ci + 1)],
                            in0=yt_h[h][0:64, 0:512],
                            scalar=0.0625,
                            in1=mbs[0:64, :],
                            op0=ALU.mult, op1=ALU.mult,
                        )
                    else:
                        yts1 = sb_w2.tile([64, 512], F16, tag="yts1", name=f"yts1_{ci}")
                        nc.vector.scalar_tensor_tensor(
                            out=yts1[:],
                            in0=yt_h[h][0:64, 0:512],
                            scalar=0.0625,
                            in1=mbs[64:128, :],
                            op0=ALU.mult, op1=ALU.mult,
                        )
                        nc.gpsimd.dma_start(
                            out=yts[64:128, 512 * ci : 512 * (ci + 1)], in_=yts1[:]
                        )
                return f

            outsb = [None]

            def piece_out(jt):
                def f():
                    if jt % 4 == 0:
                        outsb[0] = sb_o.tile(
                            [128, 4, 512], F16, tag="outsb", name=f"osb{ci}_{jt}"
                        )
                    pso = ps_q.tile([128, 512], F32, tag="psq", name=f"pso{ci}_{jt}")
                    nc.tensor.matmul(
                        pso[:],
                        wo[:, 128 * jt : 128 * (jt + 1)],
                        yts[:, 512 * ci : 512 * (ci + 1)],
                        start=True, stop=True,
                    )
                    if jt % 2 == 0:
                        nc.vector.tensor_copy(outsb[0][:, jt % 4, :], pso[:])
                    else:
                        nc.scalar.copy(outsb[0][:, jt % 4, :], pso[:])
                    if jt % 4 == 3:
                        # one batched DMA for 4 row-blocks (4KB/partition)
                        jt0 = jt - 3
                        nc.gpsimd.dma_start(
                            out=d_out[
                                128 * jt0 : 128 * (jt0 + 4),
                                512 * ci : 512 * (ci + 1),
                            ].rearrange("(c p) q -> p c q", c=4),
                            in_=outsb[0][:],
                        )
                return f

            return [piece_scale, piece_apply(0), piece_apply(1)] + [
                piece_out(jt) for jt in range(8)
            ]

        # ---------------- HAM warm-up: ~4us of junk matmuls ----------------
        pwz = ps_y.tile([128, 512], F32, tag="yt", name="pwz")
        for _w in range(6):
            nc.tensor.matmul(
                pwz[:], wz[:, 0:128], wz[:], start=True, stop=True
            )
        # preload the exp table set while the pipeline is still DMA-bound
        wze = sb_w2.tile([1, 2], F16, tag="wze", name="wze")
        nc.scalar.activation(wze[:], wz[0:1, 0:2], AF.Exp)

        # ---------------- interleaved emission ----------------
        # QKV 0-7 upfront (two chains) so the short early windows carry a
        # light DVE load; remaining tiles spread by window capacity.
        for ti in range(8):
            emit_qkv_tile(ti)
        emit_chain(0)
        emit_chain(1)
        qkv_per_window = {0: [8, 9], 1: [10, 11], 2: [12, 13, 14, 15], 3: []}
        chain_in_window = {1: 2, 2: 3}
        # out-projection of window w runs as filler in window outp_in[w]
        # (w2's DVE is already full with QKV 12-15 + chain 3; w3 is thin)
        outp_in = {0: 1, 1: 3, 2: 3}
        yts_done = {}
        for ci in range(4):
            # QKV tiles + chain first (they feed the next window's critical
            # path on DVE), then deferred out-projections
            fillers = []
            for t in qkv_per_window[ci]:
                fillers.append(lambda t=t: emit_qkv_tile(t))
            if ci in chain_in_window:
                fillers.append(lambda g=chain_in_window[ci]: emit_chain(g))
            for w, host in outp_in.items():
                if host == ci:
                    fillers = fillers + emit_scale_pieces(w, yts_done[w])
            yts_done[ci] = emit_attention(ci, fillers)
        for f in emit_scale_pieces(3, yts_done[3]):
            f()

    nc.compile()
    return nc


_NC = None


def _rope_tables():
    inv = (1.0 / 10000.0) ** (np.arange(0, HD, 2, dtype=np.float64) / HD)
    t = np.arange(T, dtype=np.float64)
    f = np.outer(t, inv)  # (T, 32)
    cc = np.concatenate([np.cos(f), np.cos(f)], axis=1).astype(np.float32)
    sc = np.concatenate([np.sin(f), -np.sin(f)], axis=1).astype(np.float32)
    return cc, sc


def kernel(x, vi, Wq, Wk, Wv, Wo, lamb, sink_weights):
    global _NC
    x = np.asarray(x, dtype=np.float32)
    vi = np.asarray(vi, dtype=np.float32)
    Wq = np.asarray(Wq, dtype=np.float32)
    Wk = np.asarray(Wk, dtype=np.float32)
    Wv = np.asarray(Wv, dtype=np.float32)
    Wo = np.asarray(Wo, dtype=np.float32)
    lam = float(np.asarray(lamb).reshape(-1)[0])
    sink = np.asarray(sink_weights, dtype=np.float32).reshape(-1)

    if _NC is None:
        _NC = _build_program()

    x0T = x[0].T  # (D, T)
    xtb = np.ascontiguousarray(
        x0T.reshape(8, 128, NT, 128).transpose(2, 1, 0, 3)
    ).astype(np.float16)  # (NT, p, i, c): xtb[ti, p, n, c] = xT[128n+p, 128ti+c]
    cc, sc = _rope_tables()
    ccb = np.ascontiguousarray(cc.reshape(NT, 128, 64).transpose(1, 0, 2)).astype(
        np.float16
    )
    scb = np.ascontiguousarray(sc.reshape(NT, 128, 64).transpose(1, 0, 2)).astype(
        np.float16
    )
    # -40 strictly above the diagonal (key > query): exp(s-40) ~ 0
    tri = (-40.0 * (np.arange(128)[:, None] > np.arange(128)[None, :])).astype(
        np.float16
    )
    idn = np.eye(128, dtype=np.float16)
    hind = np.zeros((1, 256), np.float16)
    hind[0, 0:64] = 1.0
    hind[0, 192:256] = 1.0

    in_maps = []
    for c in range(8):
        lo = 128 * c
        wqkv = np.concatenate(
            [
                Wq[lo : lo + 128].T,
                Wk[lo : lo + 128].T,
                (1.0 - lam) * Wv[lo : lo + 128].T,
            ],
            axis=1,
        )  # (D, 384)
        wqkv = np.ascontiguousarray(
            wqkv.reshape(8, 128, 384).transpose(1, 0, 2)
        ).astype(np.float16)
        esk = (np.exp(sink[2 * c : 2 * c + 2]) / 16.0).astype(np.float16).reshape(1, 2)
        in_maps.append(
            {
                "xtb": xtb,
                "wqkv": wqkv,
                "vis": np.ascontiguousarray(
                    (lam * vi[0][:, lo : lo + 128]).reshape(NT, 128, 128).transpose(1, 0, 2)
                ).astype(np.float16),
                "cc": ccb,
                "sc": scb,
                "wo": np.ascontiguousarray(Wo[:, lo : lo + 128].T).astype(np.float16),
                "tri": tri,
                "idn": idn,
                "esk": esk,
                "hind": hind,
            }
        )

    global _trace_in_maps
    _trace_in_maps = in_maps
    res = None
    for attempt in range(3):
        try:
            res = run_bass_kernel_spmd(_NC, in_maps, list(range(8)))
            break
        except Exception:
            # transient NRT_EXEC_UNIT_UNRECOVERABLE flakes have been seen on
            # the first execute after a fresh compile; retry
            if attempt == 2:
                raise
    outT = np.zeros((D, T), np.float64)
    for c in range(8):
        outT += res.results[c]["out"].astype(np.float64)
    return np.ascontiguousarray(outT.T).astype(np.float32).reshape(1, T, D)
